# revision 20
# baseline (speedup 1.0000x reference)
"""DisConv GNN message-passing kernel for 8 Trainium2 NeuronCores.

Problem: Z = l2norm(features @ W_k + b_k); 4 iterations of
  att[k] = softmax_k(mask * (Z_k Z_k^T)); Z = l2norm(Z + att @ Z)
Output: [N, K*D] channel-concat.

Strategy (row sharding, N=2048 over 8 cores, 256 rows each):
- Each core holds full replicated Z in bf16 in two layouts: ZT
  (channel-major [32c x 2048m] stacks of 4 channels) for score matmuls and
  Znm_s (m-major [128m, 16blk x 128c] per stack) for aggregation. The
  core's own 256-column f32 state never leaves the core.
- Per m-block [128m x 256n]: 8 score matmuls (D=32 contraction), exp per
  channel-stack (ScalarE), bf16 pairwise-tree denominator, approx
  reciprocal, mask*recip, broadcast multiply -> att; 8 col-packed
  aggregation matmuls accumulate over 16 m-blocks in PSUM.
- Softmax restructuring: softmax input masking is k-independent, so
  att = mask * exp(S) / sum_k exp(S) exactly matches the reference.
- l2norm via rnorm = exp(-0.5*ln(s^2+eps)) (Ln+Exp share an ACT table set).
- Boundary per round: ONE [128,512] op per stage (zsum/sq/n2/Ln/Exp/ztl),
  then TWO half AllGathers: half h carries each rank's local columns
  [h*128,(h+1)*128), i.e. the stationary data for blocks with blk%2==h.
  The block loop processes even blocks first, so AG-B's latency hides
  behind the even half's compute. The m-major aggregation layout is
  rebuilt LOCALLY from the reloaded ZT with one XBAR DMA-transpose per
  (half, stack) ([128,1024] -> [128,8,128]) -- no second collective, no
  PE transposes outside the final output round. All boundary DMAs stay on
  the SP queue: a dependency-waiting DMA head-of-line blocks its issuing
  engine's sequencer, which must not be an engine with queued compute.
- Next-block score matmuls are emitted right after the current block's
  exps, so PE's in-order queue issues them before the (att-stalled)
  aggregation matmuls and the next exps start on time.
- feat@W init matmuls and the sq/onesblk norm path run in bf16 (f32 PE
  matmul is 4x slower); costs ~3e-5 extra rel err (verified vs numpy).
- PSUM rules (HW-verified): concurrent row-group matmuls must hit
  distinct banks (4x one-bank score tiles); start_tensor_calc zeroes the
  whole 2KB bank region, so every accumulation group (per-stack agg
  tiles) needs its own bank.
"""

import sys

sys.path.insert(0, "/opt/trn_rl_repo")

import numpy as np
import ml_dtypes

N = 2048
IN_DIM = 128
K = 8
D = 32
ITERS = 4
NCORES = 8
NLOC = N // NCORES  # 256
NBLK = N // 128  # 16
EPS2 = 1e-24

BF = ml_dtypes.bfloat16

_compiled = None


DEFAULT_CFG = dict(
    score_tiles=4,   # 4 one-bank score tiles vs 2 two-bank tiles
    lvl2_eng="gpsimd",
    den_eng="gpsimd",
    rm_eng="gpsimd",
    eall_bufs=3,
    att_bufs=3,
    pipe_bufs=3,
    score_imajor=True,
    att_split=2,
    tree_split=False,
    agg_slotmajor=False,
    bf16_init=True,   # bisect: feat@W matmuls in bf16
    bf16_n2=True,     # bisect: sq/onesblk in bf16
    mask_merge=True,  # bisect: maskT as one DMA
)


def _build(reps=1, sim_mode=False, cfg=None, iters=ITERS):
    """sim_mode: single-core, collective replaced by a DRAM->DRAM DMA with
    the same dependency shape, for TimelineSim cost-model iteration."""
    import concourse.bacc as bacc
    import concourse.mybir as mybir
    from concourse import tile

    # The ACT table-load pass picks the first set containing each function,
    # which puts Exp (set 0) and Ln (set 5) in different table sets and
    # reloads tables twice per iteration boundary (~2.7us each). Restrict
    # Exp/Ln to natural_log_exp_and_others (which holds both) so one load
    # serves the whole kernel. Indices/order are preserved.
    if not getattr(bacc, "_dis_act_tables_patched", False):
        _orig_tabs = bacc.get_activation_tables

        def _patched_tabs(arch, _orig=_orig_tabs, _AF=mybir.ActivationFunctionType):
            out = {}
            for name, fns in _orig(arch).items():
                fns = set(fns)
                if name != "natural_log_exp_and_others":
                    fns.discard(_AF.Exp)
                    fns.discard(_AF.Ln)
                out[name] = fns
            return out

        bacc.get_activation_tables = _patched_tabs
        bacc._dis_act_tables_patched = True

    cfg = {**DEFAULT_CFG, **(cfg or {})}

    f32 = mybir.dt.float32
    bf16 = mybir.dt.bfloat16
    AF = mybir.ActivationFunctionType
    ALU = mybir.AluOpType

    nc = bacc.Bacc(
        "TRN2",
        target_bir_lowering=False,
        debug=False,
        num_devices=1 if sim_mode else NCORES,
    )
    nc._dis_sim_mode = sim_mode
    nc._dis_cfg = cfg

    # ---- I/O -------------------------------------------------------------
    ini_dt = bf16 if cfg["bf16_init"] else f32
    n2_dt = bf16 if cfg["bf16_n2"] else f32
    featT_loc = nc.dram_tensor("featT_loc", [IN_DIM, NLOC], ini_dt, kind="ExternalInput")
    maskT_in = nc.dram_tensor("maskT", [NBLK, 128, NLOC], bf16, kind="ExternalInput")
    wstack_in = nc.dram_tensor("wstack", [IN_DIM, K * D], ini_dt, kind="ExternalInput")
    bstack_in = nc.dram_tensor("bstack", [128, 2], f32, kind="ExternalInput")
    onesblk_in = nc.dram_tensor("onesblk", [128, 128], n2_dt, kind="ExternalInput")
    id128_in = nc.dram_tensor("id128", [128, 128], f32, kind="ExternalInput")
    out_dram = nc.dram_tensor("out", [2, 128, NLOC], f32, kind="ExternalOutput")

    rg = [list(range(NCORES))]

    with tile.TileContext(nc) as tc:
        with (
            tc.tile_pool(name="const", bufs=1) as constp,
            tc.tile_pool(name="state", bufs=2) as statep,
            tc.tile_pool(name="work", bufs=2) as workp,
            tc.tile_pool(name="psum", bufs=1, space="PSUM") as psp,
            tc.tile_pool(name="psagg", bufs=1, space="PSUM") as psaggp,
            tc.tile_pool(name="dram", bufs=2, space="DRAM") as dramp,
        ):
            # ---- persistent SBUF tensors --------------------------------
            featT = constp.tile([IN_DIM, NLOC], ini_dt)
            nc.sync.dma_start(featT[:], featT_loc[:])
            wstack = constp.tile([IN_DIM, K * D], ini_dt)
            nc.sync.dma_start(wstack[:], wstack_in[:])
            bstack = constp.tile([128, 2], f32)
            nc.sync.dma_start(bstack[:], bstack_in[:])
            onesblk = constp.tile([128, 128], n2_dt)
            nc.sync.dma_start(onesblk[:], onesblk_in[:])
            id128 = constp.tile([128, 128], f32)
            nc.sync.dma_start(id128[:], id128_in[:])
            epsb = constp.tile([128, 1], f32)
            nc.any.memset(epsb[:], EPS2)
            maskT = constp.tile([128, NBLK * NLOC], bf16)
            if cfg["mask_merge"]:
                nc.sync.dma_start(
                    maskT[:].rearrange("p (b n) -> p b n", b=NBLK),
                    maskT_in[:].rearrange("b p n -> p b n"),
                )
            else:
                for q in range(8):
                    nc.sync.dma_start(
                        maskT[:, q * 2 * NLOC : (q + 1) * 2 * NLOC].rearrange(
                            "p (b n) -> p b n", b=2
                        ),
                        maskT_in[2 * q : 2 * q + 2].rearrange("b p n -> p b n"),
                    )

            # replicated Z (bf16, rebuilt each round via two half
            # AllGathers). Half h holds blocks blk%2==h (block 2r+h = rank
            # r's local columns [h*128,(h+1)*128)), so the first collective
            # unblocks half the block loop while the second one flies.
            ZTh = [
                [constp.tile([128, N // 2], bf16, name=f"ZT{h}{s}") for s in range(2)]
                for h in range(2)
            ]
            # m-major aggregation layout: [128m, (blk//2)*128 + 32i+c]
            Znmh = [
                [constp.tile([128, N // 2], bf16, name=f"Znm{h}{s}") for s in range(2)]
                for h in range(2)
            ]

            def normalize_and_distribute(zsum, rnd, last):
                """zsum: one [128(4ch x 32c), 2*NLOC] f32 tile (stack s in
                column half s). Produces the new local f32 state (returned)
                and bf16 ZT/Znm replicas via AllGather + local transposes,
                or, if last, writes the output DRAM tensor."""
                # sq/onesblk in bf16: the n2 matmul runs at full PE rate
                # (f32 matmul is 4x slower); 32-term bf16 dot of ~unit values
                # costs ~1e-3 rel on the norm, well inside the error budget.
                sq = workp.tile([128, 2 * NLOC], n2_dt, name=f"sq{rnd}", tag="sq")
                nc.vector.tensor_tensor(sq[:], zsum[:], zsum[:], ALU.mult)
                n2tag = "sps0" if cfg["score_tiles"] == 4 else "sps0_1"
                n2 = psp.tile([128, 2 * NLOC], f32, name=f"n2{rnd}", tag=n2tag)
                nc.tensor.matmul(n2[:], onesblk[:], sq[:], start=True, stop=True)
                lg = workp.tile([128, 2 * NLOC], f32, name=f"lg{rnd}", tag="lg")
                nc.scalar.activation(lg[:], n2[:], AF.Ln, bias=epsb[:])
                rn = workp.tile([128, 2 * NLOC], f32, name=f"rn{rnd}", tag="rn", bufs=3)
                nc.scalar.activation(rn[:], lg[:], AF.Exp, scale=-0.5)
                if not last:
                    # ztl first: it gates the collectives; zloc only feeds
                    # the next round's residual add.
                    ztl = statep.tile([128, 2 * NLOC], bf16, name=f"ztl{rnd}", tag="ztl")
                    nc.vector.tensor_tensor(ztl[:], zsum[:], rn[:], ALU.mult)
                zloc = statep.tile([128, 2 * NLOC], f32, name=f"zloc{rnd}", tag="zloc")
                nc.vector.tensor_tensor(zloc[:], zsum[:], rn[:], ALU.mult)

                if last:
                    # transpose local columns to n-major for the [N, K*D]
                    # output. Row-group-concurrent PE transposes need 4
                    # distinct PSUM banks: reuse the score tags.
                    four = cfg["score_tiles"] == 4
                    for c in range(2):
                        if four:
                            pt = [
                                psp.tile([128, 64], f32, name=f"pt{rnd}{c}{i}", tag=f"sps{i}")
                                for i in range(4)
                            ]

                            def pslice(i, s, pt=pt):
                                return pt[i][:, s * 32 : s * 32 + 32]
                        else:
                            pt = [
                                psp.tile(
                                    [128, 4 * NLOC], f32, name=f"pt{rnd}{c}{h}",
                                    tag=f"sps{2 * h}_{2 * h + 1}",
                                )
                                for h in range(2)
                            ]

                            def pslice(i, s, pt=pt):
                                base = (i % 2) * 512 + s * 32
                                return pt[i // 2][:, base : base + 32]

                        for s in range(2):
                            for i in range(4):
                                nc.tensor.transpose(
                                    pslice(i, s),
                                    zloc[32 * i : 32 * (i + 1), s * NLOC + c * 128 : s * NLOC + (c + 1) * 128],
                                    id128[32 * i : 32 * (i + 1), 32 * i : 32 * (i + 1)],
                                    tile_position=(32 * i, 0),
                                )
                        ot = workp.tile([128, 256], f32, name=f"ot{c}", tag="ot")
                        for s in range(2):
                            for i in range(4):
                                k = 4 * s + i
                                nc.vector.tensor_copy(ot[:, k * 32 : (k + 1) * 32], pslice(i, s))
                        nc.sync.dma_start(out_dram[c], ot[:])
                    return zloc

                sim = getattr(nc, "_dis_sim_mode", False)
                shared = "Local" if sim else "Shared"

                # bf16 local shard, both stacks in one tile -> one DMA out.
                agins, agouts = [], []
                for h in range(2):
                    agin = dramp.tile(
                        [2, 128, 128], bf16, name=f"agin{rnd}{h}", tag=f"agin{h}"
                    )
                    agout = dramp.tile(
                        [NCORES, 2, 128, 128], bf16,
                        name=f"agout{rnd}{h}", tag=f"agout{h}", addr_space=shared,
                    )
                    agins.append(agin)
                    agouts.append(agout)
                    for s in range(2):
                        nc.sync.dma_start(
                            agin[s],
                            ztl[:, s * NLOC + h * 128 : s * NLOC + (h + 1) * 128],
                        )
                for h in range(2):
                    if sim:  # stand-in with the same dependency shape
                        for r in range(NCORES):
                            nc.sync.dma_start(agouts[h][r], agins[h][:])
                    else:
                        nc.gpsimd.collective_compute(
                            "AllGather",
                            mybir.AluOpType.bypass,
                            replica_groups=rg,
                            ins=[agins[h][:].opt()],
                            outs=[agouts[h][:].opt()],
                        )
                # per half: reload channel-major replicas (one DMA per
                # stack), then rebuild the m-major layout locally with an
                # XBAR dma-transpose (out[p, j, c] = in[c, j*128 + p]).
                # reloads/transposes stay on SP: a dependency-waiting DMA
                # head-of-line blocks its issuing engine's sequencer, and the
                # half-B reloads wait on AG-B mid-iteration -- on ACT that
                # stalled the exp stream for ~5us per iteration.
                for h in range(2):
                    for s in range(2):
                        nc.sync.dma_start(
                            ZTh[h][s][:].rearrange("p (r n) -> p r n", r=NCORES),
                            agouts[h][:, s].rearrange("r p n -> p r n"),
                        )
                    for s in range(2):
                        nc.sync.dma_start(
                            Znmh[h][s][:].rearrange("p (j c) -> p j c", c=128),
                            ZTh[h][s][:],
                            transpose=True,
                        )
                return zloc, ztl

            for rep in range(reps):
                _body_once(
                    nc, tc, tile, mybir, rep, iters,
                    featT, wstack, bstack, onesblk, id128, epsb, maskT, ZTh, Znmh,
                    statep, workp, psp, psaggp, dramp, out_dram, rg,
                    normalize_and_distribute,
                )

    nc.compile()
    return nc


def _body_once(
    nc, tc, tile, mybir, rep, iters,
    featT, wstack, bstack, onesblk, id128, epsb, maskT, ZTh, Znmh,
    statep, workp, psp, psaggp, dramp, out_dram, rg,
    normalize_and_distribute,
):
    f32 = mybir.dt.float32
    bf16 = mybir.dt.bfloat16
    AF = mybir.ActivationFunctionType
    ALU = mybir.AluOpType
    cfg = nc._dis_cfg

    # ---- init: Z0 = l2norm(features @ W + b) for local columns ---------
    # col-group matmuls (distinct output partitions) may share a PSUM bank.
    # one PSUM tile (= one accumulation zero-region/bank) per stack:
    # start_tensor_calc marks the WHOLE 2KB zero region pending-zero, so two
    # accumulation groups must never share a bank.
    ips = [
        psaggp.tile([128, NLOC], f32, name=f"initp{rep}{s}", tag=f"agg{s}")
        for s in range(2)
    ]
    for s in range(2):
        for i in range(4):
            nc.tensor.matmul(
                ips[s][32 * i : 32 * (i + 1), :],
                wstack[:, (4 * s + i) * D : (4 * s + i + 1) * D],
                featT[:],
                start=True,
                stop=True,
                tile_position=(0, 32 * i),
                skip_group_check=True,
            )
    zsum0 = workp.tile([128, 2 * NLOC], f32, name=f"zsum0_{rep}", tag="zsum")
    for s in range(2):
        nc.vector.tensor_scalar(
            zsum0[:, s * NLOC : (s + 1) * NLOC],
            ips[s][:],
            bstack[:, s : s + 1],
            None,
            ALU.add,
        )
    if iters == 0:
        normalize_and_distribute(zsum0, 0, last=True)
        return
    zloc, ztl = normalize_and_distribute(zsum0, 0, last=False)

    # ---- iterations -----------------------------------------------------
    order = (
        [(s, i) for i in range(4) for s in range(2)]
        if cfg["score_imajor"]
        else [(s, i) for s in range(2) for i in range(4)]
    )

    def make_sps(it, blk):
        # Score tiles. Concurrent row groups must hit distinct PSUM
        # banks. Channel 4s+i at E slot 2i+s ("slot order").
        if cfg["score_tiles"] == 4:
            sps = [
                psp.tile(
                    [128, 2 * NLOC], f32, name=f"sps{it}{blk}{i}", tag=f"sps{i}"
                )
                for i in range(4)
            ]

            def sslice(s, i, sps=sps):
                return sps[i][:, s * NLOC : (s + 1) * NLOC]

            exps = [(sps[i][:], i * 512) for i in range(4)]
        else:
            sps = [
                psp.tile(
                    [128, 4 * NLOC], f32, name=f"sps{it}{blk}{h}",
                    tag=f"sps{2 * h}_{2 * h + 1}",
                )
                for h in range(2)
            ]

            def sslice(s, i, sps=sps):
                base = (i % 2) * 2 * NLOC + s * NLOC
                return sps[i // 2][:, base : base + NLOC]

            exps = [(sps[h][:], h * 1024) for h in range(2)]
        return sslice, exps

    def emit_scores(sslice, blk, ztl):
        h, r = blk % 2, blk // 2
        for s, i in order:
            nc.tensor.matmul(
                sslice(s, i),
                ZTh[h][s][32 * i : 32 * (i + 1), r * 128 : (r + 1) * 128],
                ztl[32 * i : 32 * (i + 1), s * NLOC : (s + 1) * NLOC],
                start=True,
                stop=True,
                tile_position=(32 * i, 0),
            )

    blk_order = [b for b in range(NBLK) if b % 2 == 0] + [
        b for b in range(NBLK) if b % 2 == 1
    ]
    for it in range(iters):
        aggps = [
            psaggp.tile([128, NLOC], f32, name=f"agg{it}{s}", tag=f"agg{s}")
            for s in range(2)
        ]
        # scores one block ahead of the tree/agg emission: PE's in-order
        # queue then issues scores(next) before agg(blk) (which stalls on
        # att), keeping the next block's exp inputs ready just in time.
        # Even blocks first: they only need the first half AllGather.
        nxt = make_sps(it, blk_order[0])
        emit_scores(nxt[0], blk_order[0], ztl)
        for bi, blk in enumerate(blk_order):
            cur, nxt = nxt, None
            _, exps = cur
            eall = workp.tile(
                [128, K * NLOC], bf16, name=f"eall{it}{blk}", tag="eall",
                bufs=cfg["eall_bufs"],
            )
            for src_ap, col in exps:
                nc.scalar.activation(
                    eall[:, col : col + src_ap.shape[-1]], src_ap, AF.Exp
                )
            if bi + 1 < NBLK:
                nxt = make_sps(it, blk_order[bi + 1])
                emit_scores(nxt[0], blk_order[bi + 1], ztl)
            # denominator tree: 3 ops
            t1 = workp.tile([128, 1024], bf16, name=f"t1_{it}{blk}", tag="t1", bufs=cfg["pipe_bufs"])
            if cfg["tree_split"]:
                for h in range(2):
                    evh = eall[:, h * 1024 : (h + 1) * 1024].rearrange(
                        "p (a n) -> p a n", a=2
                    )
                    t1h = t1[:, h * 512 : (h + 1) * 512].rearrange(
                        "p (a n) -> p a n", a=2
                    )
                    nc.vector.tensor_tensor(
                        t1h, evh[:, :, 0:NLOC], evh[:, :, NLOC : 2 * NLOC], ALU.add
                    )
            else:
                ev = eall[:].rearrange("p (a n) -> p a n", a=4)
                t1v = t1[:].rearrange("p (a n) -> p a n", a=4)
                nc.vector.tensor_tensor(
                    t1v, ev[:, :, 0:NLOC], ev[:, :, NLOC : 2 * NLOC], ALU.add
                )
            t2 = workp.tile([128, 512], bf16, name=f"t2_{it}{blk}", tag="t2", bufs=cfg["pipe_bufs"])
            t1w = t1[:].rearrange("p (a n) -> p a n", a=2)
            t2v = t2[:].rearrange("p (a n) -> p a n", a=2)
            eng2 = nc.gpsimd if cfg["lvl2_eng"] == "gpsimd" else nc.vector
            eng2.tensor_tensor(
                t2v, t1w[:, :, 0:NLOC], t1w[:, :, NLOC : 2 * NLOC], ALU.add
            )
            den = workp.tile([128, NLOC], f32, name=f"den{it}{blk}", tag="den", bufs=cfg["pipe_bufs"])
            engd = nc.gpsimd if cfg["den_eng"] == "gpsimd" else nc.vector
            engd.tensor_tensor(
                den[:], t2[:, 0:NLOC], t2[:, NLOC : 2 * NLOC], ALU.add
            )
            rcp = workp.tile([128, NLOC], f32, name=f"rcp{it}{blk}", tag="rcp", bufs=cfg["pipe_bufs"])
            nc.vector.reciprocal_approx_fast(rcp[:], den[:])
            rmask = workp.tile([128, NLOC], bf16, name=f"rm{it}{blk}", tag="rm", bufs=cfg["pipe_bufs"])
            engr = nc.gpsimd if cfg["rm_eng"] == "gpsimd" else nc.vector
            engr.tensor_tensor(
                rmask[:], rcp[:], maskT[:, blk * NLOC : (blk + 1) * NLOC], ALU.mult
            )
            att = workp.tile(
                [128, K * NLOC], bf16, name=f"att{it}{blk}", tag="att", bufs=cfg["att_bufs"]
            )
            nsp = cfg["att_split"]
            kk = K // nsp
            for h in range(nsp):
                lo = h * kk * NLOC
                hi = (h + 1) * kk * NLOC
                nc.vector.tensor_tensor(
                    att[:, lo:hi].rearrange("p (a n) -> p a n", a=kk),
                    eall[:, lo:hi].rearrange("p (a n) -> p a n", a=kk),
                    rmask[:, None, :].to_broadcast((128, kk, NLOC)),
                    ALU.mult,
                )
            if cfg["agg_slotmajor"]:
                agord = [(slot % 2, slot // 2) for slot in range(8)]
            else:
                agord = [(s, i) for s in range(2) for i in range(4)]
            h, r = blk % 2, blk // 2
            for s, i in agord:
                slot = 2 * i + s  # channel 4s+i in replica layouts
                nc.tensor.matmul(
                    aggps[s][32 * i : 32 * (i + 1), :],
                    Znmh[h][s][:, r * 128 + i * D : r * 128 + (i + 1) * D],
                    att[:, slot * NLOC : (slot + 1) * NLOC],
                    start=(bi == 0),
                    stop=(bi == NBLK - 1),
                    tile_position=(0, 32 * i),
                    skip_group_check=True,
                )
        # residual + renorm + redistribute
        zsum = workp.tile([128, 2 * NLOC], f32, name=f"zsum{it}", tag="zsum")
        for s in range(2):
            nc.vector.tensor_tensor(
                zsum[:, s * NLOC : (s + 1) * NLOC],
                zloc[:, s * NLOC : (s + 1) * NLOC],
                aggps[s][:],
                ALU.add,
            )
        if it == iters - 1:
            normalize_and_distribute(zsum, it + 1, last=True)
        else:
            zloc, ztl = normalize_and_distribute(zsum, it + 1, last=False)


def _prep_inputs(adj, features, W, b, cfg=None):
    cfg = {**DEFAULT_CFG, **(cfg or {})}
    adj = np.asarray(adj)
    features = np.asarray(features, np.float32)
    W = np.asarray(W, np.float32)
    b = np.asarray(b, np.float32)

    wstack = np.ascontiguousarray(W.transpose(1, 0, 2).reshape(IN_DIM, K * D)).astype(BF if cfg['bf16_init'] else np.float32)
    bstack = np.zeros((128, 2), np.float32)
    for s in range(2):
        for i in range(4):
            bstack[32 * i : 32 * (i + 1), s] = b[4 * s + i]
    onesblk = np.zeros((128, 128), BF if cfg['bf16_n2'] else np.float32)
    for j in range(4):
        onesblk[32 * j : 32 * (j + 1), 32 * j : 32 * (j + 1)] = 1.0
    id128 = np.eye(128, dtype=np.float32)

    in_maps = []
    for c in range(NCORES):
        rows = slice(c * NLOC, (c + 1) * NLOC)
        featT_loc = np.ascontiguousarray(features[rows].T).astype(BF if cfg['bf16_init'] else np.float32)
        maskT = (adj[rows].T > 0).astype(np.float32).astype(BF)
        maskT = np.ascontiguousarray(maskT.reshape(NBLK, 128, NLOC))
        in_maps.append(
            {
                "featT_loc": featT_loc,
                "maskT": maskT,
                "wstack": wstack,
                "bstack": bstack,
                "onesblk": onesblk,
                "id128": id128,
            }
        )
    return in_maps


def run(adj, features, W, b, trace=False, **trace_kwargs):
    global _compiled
    if _compiled is None:
        _compiled = _build()
    from concourse import bass_utils

    in_maps = _prep_inputs(adj, features, W, b)
    res = bass_utils.run_bass_kernel_spmd(
        _compiled, in_maps, core_ids=list(range(NCORES)), trace=trace, **trace_kwargs
    )
    outs = [res.results[c]["out"].reshape(NLOC, NLOC) for c in range(NCORES)]
    full = np.concatenate(outs, axis=0)
    return full, res


def kernel(adj, features, W, b):
    full, _ = run(adj, features, W, b, trace=False)
    return full
